# revision 1
# baseline (speedup 1.0000x reference)
"""GQA with sliding-window + ALiBi (reduces to banded causal attention) on 8 TRN2 cores.

Sharding: 8 cores = 2 batches x 4 kv-head groups. Each core computes, for its
(batch b, kv group gi): Q projection for its 4 query heads, K/V projection for
its 1 kv head, banded sliding-window attention (window 1024, causal), and a
partial row-parallel Wo matmul. Host sums the 4 partials per batch.

Math notes (exact reductions of the reference):
- ALiBi bias is -clip(j-i,0)*slope: zero on all causal positions, nonzero only
  where the causal mask kills the score -> drop it entirely.
- The sliding mask adds +1.0 uniformly inside the window: softmax-invariant.
- Out-of-window/causal positions get -1e9 -> exp underflows to exactly 0.
- Scores are O(1), so softmax without max-subtraction is safe in fp32.
All matmuls run as float32r (measured bit-identical to fp32 on TRN2 HW, 4x rate).
"""
import math
from contextlib import ExitStack

import numpy as np

import concourse.tile as tile
from concourse import bacc, mybir
from concourse.bass_utils import run_bass_kernel_spmd
from concourse.masks import make_identity

dt = mybir.dt

B, S, H = 2, 2048, 2048
NUM_HEADS, KV_HEADS, D = 16, 4, 128
WINDOW = 1024
GH = 4            # query heads per kv head (per core)
GD = GH * D       # 512: per-core slice of the hidden dim
SCALE = 1.0 / math.sqrt(D)
NEG = -1e9
QB = 256          # query columns per attention group (2 blocks of 128)
NG = S // QB      # 8 query groups
KT = H // 128     # 16 contraction tiles for projections

_nc_cache = None


def _build_nc(ptp_bufs=2, hstp_bufs=2, vtp_bufs=2, gh_order='hg', phases=3):
    nc = bacc.Bacc()
    hsT = nc.declare_dram_parameter("hsT", [4, KT, 128, 512], dt.float32r, isOutput=False)
    wq = nc.declare_dram_parameter("wq", [H, GD], dt.float32r, isOutput=False)
    wk = nc.declare_dram_parameter("wk", [H, D], dt.float32r, isOutput=False)
    wv = nc.declare_dram_parameter("wv", [H, D], dt.float32r, isOutput=False)
    wo = nc.declare_dram_parameter("wo", [GD, H], dt.float32r, isOutput=False)
    masks = nc.declare_dram_parameter("masks", [4, 128, QB], dt.float32, isOutput=False)
    out = nc.declare_dram_parameter("out", [16, 4, 128, 512], dt.float32, isOutput=True)

    with tile.TileContext(nc) as tc, ExitStack() as ctx:
        consts = ctx.enter_context(tc.tile_pool(name="consts", bufs=1))
        wpool = ctx.enter_context(tc.tile_pool(name="wpool", bufs=1))
        big = ctx.enter_context(tc.tile_pool(name="big", bufs=1))
        hstp = ctx.enter_context(tc.tile_pool(name="hstp", bufs=hstp_bufs))
        vtp = ctx.enter_context(tc.tile_pool(name="vtp", bufs=vtp_bufs))
        ptp = ctx.enter_context(tc.tile_pool(name="ptp", bufs=ptp_bufs))
        smalls = ctx.enter_context(tc.tile_pool(name="smalls", bufs=4))
        outp = ctx.enter_context(tc.tile_pool(name="outp", bufs=4))
        psum = ctx.enter_context(tc.tile_pool(name="psum", bufs=8, space="PSUM"))

        # constants
        ident32 = consts.tile([128, 128], dt.float32)
        make_identity(nc, ident32)
        ident = consts.tile([128, 128], dt.float32r)
        nc.vector.tensor_copy(ident, ident32)
        ones32 = consts.tile([128, 128], dt.float32)
        nc.vector.memset(ones32, 1.0)
        ones = consts.tile([128, 128], dt.float32r)
        nc.vector.tensor_copy(ones, ones32)
        # weights: tiles declared here, DMAs issued inside chunk-0 loop so the
        # queue order interleaves weights with the first hst tiles
        wq_t = [wpool.tile([128, GD], dt.float32r, tag=f"wq{t}", name=f"wq{t}")
                for t in range(KT)]
        wk_t = [wpool.tile([128, D], dt.float32r, tag=f"wk{t}", name=f"wk{t}")
                for t in range(KT)]
        wv_t = [wpool.tile([128, D], dt.float32r, tag=f"wv{t}", name=f"wv{t}")
                for t in range(KT)]
        # persistent activations
        qT = [big.tile([128, S], dt.float32r, tag=f"qT{h}", name=f"qT{h}") for h in range(GH)]
        kT = big.tile([128, S], dt.float32r, tag="kT")
        v = big.tile([128, S], dt.float32r, tag="v")
        ohT = [big.tile([128, S], dt.float32r, tag=f"ohT{h}", name=f"ohT{h}") for h in range(GH)]

        # ---- Phase 1: projections (per 512-wide s-chunk) ----
        for ch in range(4):
            q_ps = [psum.tile([128, 512], dt.float32, tag="ps", name=f"qps{ch}_{h}") for h in range(GH)]
            k_ps = psum.tile([128, 512], dt.float32, tag="ps")
            v_ps = psum.tile([128, 512], dt.float32, tag="ps")
            for t in range(KT):
                if ch == 0:
                    nc.sync.dma_start(out=wq_t[t], in_=wq[t * 128:(t + 1) * 128, :])
                    nc.sync.dma_start(out=wk_t[t], in_=wk[t * 128:(t + 1) * 128, :])
                    nc.sync.dma_start(out=wv_t[t], in_=wv[t * 128:(t + 1) * 128, :])
                hst = hstp.tile([128, 512], dt.float32r, tag="hst")
                nc.sync.dma_start(out=hst, in_=hsT[ch, t])
                st = (t == 0)
                sp = (t == KT - 1)
                for h in range(GH):
                    nc.tensor.matmul(q_ps[h], lhsT=wq_t[t][:, h * 128:(h + 1) * 128],
                                     rhs=hst, start=st, stop=sp)
                nc.tensor.matmul(k_ps, lhsT=wk_t[t], rhs=hst, start=st, stop=sp)
                nc.tensor.matmul(v_ps, lhsT=wv_t[t], rhs=hst, start=st, stop=sp)
            for h in range(GH):
                nc.vector.tensor_copy(qT[h][:, ch * 512:(ch + 1) * 512], q_ps[h])
            nc.vector.tensor_copy(kT[:, ch * 512:(ch + 1) * 512], k_ps)
            vt = vtp.tile([128, 512], dt.float32r, tag="vt")
            nc.vector.tensor_copy(vt, v_ps)
            for j in range(4):
                tp = psum.tile([128, 128], dt.float32r, tag="ps")
                nc.tensor.transpose(tp, vt[:, j * 128:(j + 1) * 128], ident)
                nc.scalar.copy(
                    v[:, (4 * ch + j) * 128:(4 * ch + j + 1) * 128], tp)

        # deferred loads: needed only from attention/Wo onward
        mask_t = []
        for i in range(4):
            mt = consts.tile([128, QB], dt.float32, tag=f"mask{i}", name=f"mask{i}")
            nc.sync.dma_start(out=mt, in_=masks[i])
            mask_t.append(mt)
        wo_t = []
        for ct in range(4):
            wot = wpool.tile([128, H], dt.float32r, tag=f"wo{ct}", name=f"wo{ct}")
            nc.sync.dma_start(out=wot, in_=wo[ct * 128:(ct + 1) * 128, :])
            wo_t.append(wot)

        # ---- Phase 2: banded attention, scores transposed (S^T[k, q]) ----
        if phases < 2:
            for st in range(16):
                nc.sync.dma_start(out=out[st], in_=kT[:, :H].bitcast(dt.float32).rearrange("p (e n) -> e p n", e=4))
        mask_for_o = {1: 1, 0: 0, -7: 3, -8: 2}
        hg_pairs = ([(h, g) for h in range(GH) for g in range(NG)]
                    if gh_order == 'hg' else
                    [(h, g) for g in range(NG) for h in range(GH)])
        if phases < 2:
            hg_pairs = []
        for h, g in hg_pairs:
            if True:
                kjs = list(range(max(0, 2 * g - 8), 2 * g + 2))
                av = psum.tile([128, QB], dt.float32, tag="ps")
                den = psum.tile([1, QB], dt.float32, tag="ps")
                batches = [kjs[i:i + 2] for i in range(0, len(kjs), 2)]
                for bi, bk in enumerate(batches):
                    sps = psum.tile([128, QB * len(bk)], dt.float32, tag="ps")
                    for idx, kj in enumerate(bk):
                        nc.tensor.matmul(
                            sps[:, idx * QB:(idx + 1) * QB],
                            lhsT=kT[:, kj * 128:(kj + 1) * 128],
                            rhs=qT[h][:, g * QB:(g + 1) * QB],
                            start=True, stop=True)
                        mi = mask_for_o.get(kj - 2 * g)
                        if mi is not None:
                            nc.vector.tensor_add(
                                sps[:, idx * QB:(idx + 1) * QB],
                                sps[:, idx * QB:(idx + 1) * QB], mask_t[mi])
                    pt = ptp.tile([128, QB * 2], dt.float32r, tag="pt")
                    nc.scalar.activation(
                        pt[:, :QB * len(bk)], sps,
                        mybir.ActivationFunctionType.Exp, scale=SCALE)
                    for idx, kj in enumerate(bk):
                        first = (bi == 0 and idx == 0)
                        last = (kj == kjs[-1])
                        nc.tensor.matmul(
                            den, lhsT=ones[:, 0:1],
                            rhs=pt[:, idx * QB:(idx + 1) * QB],
                            start=first, stop=last)
                        nc.tensor.matmul(
                            av, lhsT=v[:, kj * 128:(kj + 1) * 128],
                            rhs=pt[:, idx * QB:(idx + 1) * QB],
                            start=first, stop=last)
                rc = smalls.tile([1, QB], dt.float32r, tag="rc")
                with nc.allow_low_precision(reason="f32r is full fp32 bits"):
                    nc.vector.reciprocal(rc, den)
                bc = psum.tile([128, QB], dt.float32, tag="ps")
                nc.tensor.matmul(bc, lhsT=ones[0:1, :], rhs=rc, start=True, stop=True)
                bcs = smalls.tile([128, QB], dt.float32, tag="bcs")
                nc.scalar.copy(bcs, bc)
                nc.vector.tensor_mul(ohT[h][:, g * QB:(g + 1) * QB], av, bcs)

        # ---- Phase 3: partial Wo (row-parallel) ----
        for st in range(16 if phases >= 3 else 0):
            for e in range(4):
                wops = psum.tile([128, 512], dt.float32, tag="ps")
                for ct in range(4):
                    nc.tensor.matmul(
                        wops, lhsT=ohT[ct][:, st * 128:(st + 1) * 128],
                        rhs=wo_t[ct][:, e * 512:(e + 1) * 512],
                        start=(ct == 0), stop=(ct == 3))
                osb = outp.tile([128, 512], dt.float32, tag="osb")
                nc.scalar.copy(osb, wops)
                nc.sync.dma_start(out=out[st, e], in_=osb)
        if phases == 2:
            for st2 in range(4):
                nc.sync.dma_start(out=out[st2], in_=ohT[st2].bitcast(dt.float32).rearrange("p (e n) -> e p n", e=4))

    nc.compile()
    return nc


def _build_masks():
    kk = np.arange(128)[:, None]
    qq = np.arange(128)[None, :]
    diag = np.where(kk <= qq, 0.0, NEG).astype(np.float32)
    edge = np.where(kk >= qq, 0.0, NEG).astype(np.float32)
    full = np.full((128, 128), NEG, np.float32)
    none = np.zeros((128, 128), np.float32)
    return np.stack([
        np.hstack([diag, none]),   # o = 0
        np.hstack([full, diag]),   # o = +1
        np.hstack([edge, full]),   # o = -8
        np.hstack([none, edge]),   # o = -7
    ])


def kernel(hidden_states, Wq, Wk, Wv, Wo):
    global _nc_cache
    if _nc_cache is None:
        _nc_cache = _build_nc()
    nc = _nc_cache

    masks = _build_masks()
    hsT = []
    for b in range(B):
        ht = np.ascontiguousarray(hidden_states[b].T)                 # [H, S]
        t4 = ht.reshape(KT, 128, 4, 512).transpose(2, 0, 1, 3)        # [ch, t, 128, 512]
        hsT.append(np.ascontiguousarray(t4))
    in_maps = []
    for b in range(B):
        for gi in range(KV_HEADS):
            in_maps.append({
                "hsT": hsT[b],
                "wq": np.ascontiguousarray(Wq[:, gi * GD:(gi + 1) * GD]),
                "wk": np.ascontiguousarray(Wk[:, gi * D:(gi + 1) * D]),
                "wv": np.ascontiguousarray(Wv[:, gi * D:(gi + 1) * D]),
                "wo": np.ascontiguousarray(Wo[gi * GD:(gi + 1) * GD, :]),
                "masks": masks,
            })
    res = run_bass_kernel_spmd(nc, in_maps, list(range(8)))
    out = np.zeros((B, S, H), np.float32)
    for b in range(B):
        acc = None
        for gi in range(KV_HEADS):
            o = res.results[b * KV_HEADS + gi]["out"]
            acc = o.copy() if acc is None else acc + o
        out[b] = acc.transpose(0, 2, 1, 3).reshape(S, H)              # [16,4,128,512] -> [S,H]
    return out



# revision 33
# speedup vs baseline: 1.9920x; 1.9920x over previous
"""GQA with sliding-window + ALiBi (reduces to banded causal attention) on 8 TRN2 cores.

Sharding: 8 cores = 2 batches x 4 kv-head groups. Each core computes, for its
(batch b, kv group gi): Q projection for its 4 query heads, K/V projection for
its 1 kv head, banded sliding-window attention (window 1024, causal), and a
partial row-parallel Wo matmul. Host sums the 4 partials per batch.

Math notes (exact reductions of the reference):
- ALiBi bias is -clip(j-i,0)*slope: zero on all causal positions, nonzero only
  where the causal mask kills the score -> drop it entirely.
- The sliding mask adds +1.0 uniformly inside the window: softmax-invariant.
- Out-of-window/causal positions are exactly zeroed by multiplying exp(score)
  with a 0/1 mask (scores are O(1) so exp never overflows).
- Scores are O(1), so softmax without max-subtraction is safe.

Implementation notes (v1, bf16):
- All activations/weights stream as bf16 (halves DMA + SBUF; PE rate is the
  same 1 cycle/row as fp32r, accumulation stays fp32 in PSUM).
- Big batched DMAs (one per weight tensor / hsT chunk) to amortize the shared
  HWDGE descriptor stage.
- Attention is software-pipelined: score matmuls + exp run 2 quads ahead of
  the AV/denominator matmuls so PE never waits on the Act engine.
- Wo is interleaved one query-group behind attention, spreading its PSUM
  drain + output DMA across the attention phase.
- PSUM->SBUF drains are spread across DVE/Act/Pool engines.
"""
import math
from contextlib import ExitStack

import numpy as np
import ml_dtypes

import concourse.tile as tile
from concourse import bacc, mybir
from concourse.bass_utils import run_bass_kernel_spmd
from concourse.masks import make_identity

dt = mybir.dt
BF16 = ml_dtypes.bfloat16

B, S, H = 2, 2048, 2048
NUM_HEADS, KV_HEADS, D = 16, 4, 128
WINDOW = 1024
GH = 4            # query heads per kv head (per core)
GD = GH * D       # 512: per-core slice of the hidden dim
SCALE = 1.0 / math.sqrt(D)
QB = 256          # query columns per attention group
NG = S // QB      # 8 query groups
KT = H // 128     # 16 contraction tiles for projections

_nc_cache = None


def _build_nc(depth=3):
    nc = bacc.Bacc()
    hsT = nc.declare_dram_parameter("hsT", [4, KT, 128, 512], dt.bfloat16, isOutput=False)
    wq = nc.declare_dram_parameter("wq", [KT, 128, GD], dt.bfloat16, isOutput=False)
    wkv = nc.declare_dram_parameter("wkv", [KT, 128, 2 * D], dt.bfloat16, isOutput=False)
    wo = nc.declare_dram_parameter("wo", [4, 128, H], dt.bfloat16, isOutput=False)
    masks = nc.declare_dram_parameter("masks", [128, 4 * QB], dt.bfloat16, isOutput=False)
    out = nc.declare_dram_parameter("out", [16, 4, 128, 512], dt.bfloat16, isOutput=True)

    with tile.TileContext(nc) as tc, ExitStack() as ctx:
        consts = ctx.enter_context(tc.tile_pool(name="consts", bufs=1))
        wpool = ctx.enter_context(tc.tile_pool(name="wpool", bufs=1))
        big = ctx.enter_context(tc.tile_pool(name="big", bufs=1))
        hstp = ctx.enter_context(tc.tile_pool(name="hstp", bufs=2))
        vtp = ctx.enter_context(tc.tile_pool(name="vtp", bufs=2))
        ptp = ctx.enter_context(tc.tile_pool(name="ptp", bufs=5))
        smalls = ctx.enter_context(tc.tile_pool(name="smalls", bufs=3))
        outp = ctx.enter_context(tc.tile_pool(name="outp", bufs=3))

        # constants
        ident32 = consts.tile([128, 128], dt.float32)
        make_identity(nc, ident32)
        ident = consts.tile([128, 128], dt.float32r)
        nc.vector.tensor_copy(ident, ident32)
        ones32 = consts.tile([128, 128], dt.float32)
        nc.vector.memset(ones32, 1.0)
        ones_bf = consts.tile([128, 128], dt.bfloat16)
        nc.vector.tensor_copy(ones_bf, ones32)
        mask_t = consts.tile([128, 4 * QB], dt.bfloat16)

        # weights (single big SBUF tiles, loaded with few big DMAs)
        wq_sb = wpool.tile([128, KT * GD], dt.bfloat16)      # 16KB/part
        wkv_sb = wpool.tile([128, KT * 2 * D], dt.bfloat16)  # 8KB/part
        wo_sb = wpool.tile([128, 4 * H], dt.bfloat16)        # 16KB/part

        # persistent activations (bf16)
        qT = [big.tile([128, S], dt.bfloat16, name=f"qT{h}") for h in range(GH)]
        kT = big.tile([128, S], dt.bfloat16)
        v = big.tile([128, S], dt.bfloat16)   # [key, d] layout per 128-block
        ohT = [big.tile([128, S], dt.bfloat16, name=f"ohT{h}") for h in range(GH)]

        wq_v = wq_sb.rearrange("p (t n) -> p t n", t=KT)
        wkv_v = wkv_sb.rearrange("p (t n) -> p t n", t=KT)

        # ---- Phase 1: projections (per 512-wide s-chunk) ----
        with tc.tile_pool(name="psA", bufs=8, space="PSUM") as psA:
            hst_tiles = []
            for ch in range(4):
                hst = hstp.tile([128, KT * 512], dt.bfloat16, tag="hst", name=f"hst{ch}")
                hst_tiles.append(hst)
            # chunk 0: quarter-granularity DMAs interleaved with weight quarters
            h0v = hst_tiles[0].rearrange("p (t n) -> p t n", t=KT)
            for sl in (slice(0, 2), slice(2, 4), slice(4, 8),
                       slice(8, 12), slice(12, 16)):
                nc.sync.dma_start(out=wq_v[:, sl], in_=wq[sl].rearrange("t p n -> p t n"))
                nc.sync.dma_start(out=wkv_v[:, sl], in_=wkv[sl].rearrange("t p n -> p t n"))
                nc.sync.dma_start(out=h0v[:, sl], in_=hsT[0, sl].rearrange("t p n -> p t n"))

            for ch in range(4):
                if ch + 1 < 4:
                    nxt = hst_tiles[ch + 1]
                    nc.sync.dma_start(
                        out=nxt.rearrange("p (t n) -> p t n", t=KT),
                        in_=hsT[ch + 1].rearrange("t p n -> p t n"))
                if ch == 0:
                    nc.sync.dma_start(out=mask_t, in_=masks[:, :])
                    nc.sync.dma_start(
                        out=wo_sb.rearrange("p (c n) -> p c n", c=4),
                        in_=wo[:].rearrange("c p n -> p c n"))
                hst = hst_tiles[ch]
                q_ps = [psA.tile([128, 512], dt.float32, tag="ps", name=f"qps{ch}_{h}")
                        for h in range(GH)]
                k_ps = psA.tile([128, 512], dt.float32, tag="ps")
                v_ps = psA.tile([128, 512], dt.float32, tag="ps")

                def vtrans(ch, vt):
                    # transpose V of a finished chunk, interleaved into the
                    # next chunk's matmul stream so PE never waits on it
                    for j in range(4):
                        tp = psA.tile([128, 128], dt.float32r, tag="ps",
                                      name=f"tp{ch}_{j}")
                        nc.tensor.transpose(tp, vt[:, j * 128:(j + 1) * 128], ident)
                        nc.scalar.copy(
                            v[:, (4 * ch + j) * 128:(4 * ch + j + 1) * 128], tp)

                for t in range(KT):
                    if ch > 0 and t == 4:
                        vtrans(ch - 1, prev_vt)
                    rhs = hst[:, t * 512:(t + 1) * 512]
                    st = (t == 0)
                    sp = (t == KT - 1)
                    for h in range(GH):
                        nc.tensor.matmul(
                            q_ps[h], lhsT=wq_sb[:, t * 512 + h * 128: t * 512 + (h + 1) * 128],
                            rhs=rhs, start=st, stop=sp)
                    nc.tensor.matmul(k_ps, lhsT=wkv_sb[:, t * 256: t * 256 + 128],
                                     rhs=rhs, start=st, stop=sp)
                    nc.tensor.matmul(v_ps, lhsT=wkv_sb[:, t * 256 + 128: t * 256 + 256],
                                     rhs=rhs, start=st, stop=sp)
                # drain PSUM on three engines in slot-rotation order
                cs = slice(ch * 512, (ch + 1) * 512)
                nc.vector.tensor_copy(qT[0][:, cs], q_ps[0])
                nc.scalar.copy(qT[1][:, cs], q_ps[1])
                nc.vector.tensor_copy(qT[2][:, cs], q_ps[2])
                nc.vector.tensor_copy(qT[3][:, cs], q_ps[3])
                nc.scalar.copy(kT[:, cs], k_ps)
                vt = vtp.tile([128, 512], dt.float32r, tag="vt")
                nc.scalar.copy(vt, v_ps)
                prev_vt = vt
            vtrans(3, prev_vt)

        # ---- Phase 2+3: banded attention (S^T[k,q] layout) + interleaved Wo ----
        mask_R = mask_t[:, 0:512]
        mask_L = mask_t[:, 512:1024]
        with tc.tile_pool(name="psB", bufs=1, space="PSUM") as psB:
            pending = []
            fin_done = set()

            def drain(n):
                while len(pending) > n:
                    pending.pop(0)()

            wo_parts = []  # deferred per-(st,e) Wo emission closures

            def mk_wo(st, e, osb):
                def f():
                    wop = psB.tile([128, 512], dt.float32, tag="wop", bufs=2,
                                   name=f"wop{st}_{e}")
                    for ct in range(4):
                        nc.tensor.matmul(
                            wop, lhsT=ohT[ct][:, st * 128:(st + 1) * 128],
                            rhs=wo_sb[:, ct * 2048 + e * 512: ct * 2048 + (e + 1) * 512],
                            start=(ct == 0), stop=(ct == 3))
                    if e % 2 == 0:
                        nc.scalar.copy(osb[:, e * 512:(e + 1) * 512], wop)
                    else:
                        nc.vector.tensor_copy(osb[:, e * 512:(e + 1) * 512], wop)
                    nc.sync.dma_start(
                        out=out[st, e], in_=osb[:, e * 512:(e + 1) * 512])
                return f

            def queue_wo(g):
                for st in (2 * g, 2 * g + 1):
                    osb = outp.tile([128, 2048], dt.bfloat16, tag="osb", name=f"osb{st}")
                    for e in range(4):
                        wo_parts.append(mk_wo(st, e, osb))

            for g in range(NG):
                if g >= 1:
                    queue_wo(g - 1)
                for h in range(GH):
                    kjs = list(range(max(0, 2 * g - 8), 2 * g + 2))
                    prs = [kjs[i:i + 2] for i in range(0, len(kjs), 2)]
                    nb = len(prs)
                    av = psB.tile([128, QB], dt.float32, tag="av", bufs=2,
                                  name=f"av{h}_{g}")
                    ptsum = None
                    prev_pt = None
                    for bi, pr in enumerate(prs):
                        sps = psB.tile([128, 512], dt.float32, tag="sps", bufs=3,
                                       name=f"sps{h}_{g}_{bi}")
                        for idx, kj in enumerate(pr):
                            nc.tensor.matmul(
                                sps[:, idx * QB:(idx + 1) * QB],
                                lhsT=kT[:, kj * 128:(kj + 1) * 128],
                                rhs=qT[h][:, g * QB:(g + 1) * QB],
                                start=True, stop=True)
                        pt = ptp.tile([128, 512], dt.bfloat16, tag="pt",
                                      name=f"pt{h}_{g}_{bi}")
                        nc.scalar.activation(
                            pt, sps, mybir.ActivationFunctionType.Exp, scale=SCALE)
                        if bi == nb - 1:
                            nc.vector.tensor_mul(pt, pt, mask_R)
                        elif bi == 0 and g >= 4:
                            nc.vector.tensor_mul(pt, pt, mask_L)
                        if bi == 1:
                            ptsum = smalls.tile([128, 512], dt.bfloat16, tag="ptsum",
                                                name=f"ptsum{h}_{g}")

                        def mk_av(pt=pt, pr=pr, bi=bi, first=(bi == 0),
                                  last=(bi == nb - 1), av=av, ptsum=ptsum,
                                  prev_pt=prev_pt):
                            def f():
                                for idx, kj in enumerate(pr):
                                    nc.tensor.matmul(
                                        av, lhsT=v[:, kj * 128:(kj + 1) * 128],
                                        rhs=pt[:, idx * QB:(idx + 1) * QB],
                                        start=(first and idx == 0),
                                        stop=(last and idx == len(pr) - 1))
                                # running pt-sum (softmax denominator); first
                                # add on the otherwise-idle Pool engine
                                if bi == 1:
                                    nc.gpsimd.tensor_add(ptsum, prev_pt, pt)
                                elif bi > 1:
                                    nc.vector.tensor_add(ptsum, ptsum, pt)
                            return f
                        pending.append(mk_av())
                        drain(depth)
                        prev_pt = pt

                    def mk_fin(h=h, g=g, av=av, ptsum=ptsum, pt=pt):
                        def f():
                            # fold the two kj-halves -> per-q key-sums [128, 256]
                            src = ptsum if ptsum is not None else pt
                            ptf = smalls.tile([128, QB], dt.bfloat16, tag="ptf",
                                              name=f"ptf{h}_{g}")
                            nc.vector.tensor_add(ptf, src[:, 0:QB], src[:, QB:2 * QB])
                            denb = psB.tile([128, QB], dt.float32, tag="den", bufs=1,
                                            name=f"den{h}_{g}")
                            nc.tensor.matmul(denb, lhsT=ones_bf, rhs=ptf,
                                             start=True, stop=True)
                            rcb = smalls.tile([128, QB], dt.float32r, tag="bcs",
                                              name=f"rcb{h}_{g}")
                            with nc.allow_low_precision(reason="f32r is full fp32 bits"):
                                nc.vector.reciprocal(rcb, denb)
                            nc.vector.tensor_mul(
                                ohT[h][:, g * QB:(g + 1) * QB], av, rcb)
                            fin_done.add((h, g))
                        return f
                    pending.append(mk_fin())
                    # interleave two Wo pieces of the previous group; their
                    # ohT inputs must have been written (fins emitted) first
                    if g >= 1:
                        while (3, g - 1) not in fin_done and pending:
                            pending.pop(0)()
                        for _ in range(2):
                            if wo_parts:
                                wo_parts.pop(0)()
            drain(0)
            queue_wo(NG - 1)
            while wo_parts:
                wo_parts.pop(0)()

    nc.compile()
    return nc


def _build_masks():
    kk = np.arange(128)[:, None]
    qq = np.arange(256)[None, :]
    # right-edge pair (kj=2g, 2g+1): causal edge
    r0 = (kk <= qq).astype(np.float32)             # o = 0
    r1 = (kk + 128 <= qq).astype(np.float32)       # o = +1
    # left-edge pair (kj=2g-8, 2g-7): window edge
    l0 = (kk >= qq).astype(np.float32)             # o = -8
    l1 = (kk + 128 >= qq).astype(np.float32)       # o = -7
    return np.hstack([r0, r1, l0, l1]).astype(BF16)  # [128, 1024]


def kernel(hidden_states, Wq, Wk, Wv, Wo):
    global _nc_cache
    if _nc_cache is None:
        _nc_cache = _build_nc()
    nc = _nc_cache

    masks = _build_masks()
    hsT = []
    for b in range(B):
        ht = np.ascontiguousarray(hidden_states[b].T)                 # [H, S]
        t4 = ht.reshape(KT, 128, 4, 512).transpose(2, 0, 1, 3)        # [ch, t, 128, 512]
        hsT.append(np.ascontiguousarray(t4).astype(BF16))
    in_maps = []
    for b in range(B):
        for gi in range(KV_HEADS):
            wkv = np.concatenate(
                [Wk[:, gi * D:(gi + 1) * D], Wv[:, gi * D:(gi + 1) * D]], axis=1)
            in_maps.append({
                "hsT": hsT[b],
                "wq": Wq[:, gi * GD:(gi + 1) * GD].reshape(KT, 128, GD).astype(BF16),
                "wkv": wkv.reshape(KT, 128, 2 * D).astype(BF16),
                "wo": Wo[gi * GD:(gi + 1) * GD, :].reshape(4, 128, H).astype(BF16),
                "masks": masks,
            })
    res = run_bass_kernel_spmd(nc, in_maps, list(range(8)))
    out = np.zeros((B, S, H), np.float32)
    for b in range(B):
        acc = None
        for gi in range(KV_HEADS):
            o = np.asarray(res.results[b * KV_HEADS + gi]["out"]).astype(np.float32)
            acc = o if acc is None else acc + o
        out[b] = acc.transpose(0, 2, 1, 3).reshape(S, H)              # [16,4,128,512] -> [S,H]
    return out


# revision 52
# speedup vs baseline: 1.9941x; 1.0011x over previous
"""GQA with sliding-window + ALiBi (reduces to banded causal attention) on 8 TRN2 cores.

Sharding: 8 cores = 2 batches x 4 kv-head groups. Each core computes, for its
(batch b, kv group gi): Q projection for its 4 query heads, K/V projection for
its 1 kv head, banded sliding-window attention (window 1024, causal), and a
partial row-parallel Wo matmul. Host sums the 4 partials per batch.

Math notes (exact reductions of the reference):
- ALiBi bias is -clip(j-i,0)*slope: zero on all causal positions, nonzero only
  where the causal mask kills the score -> drop it entirely.
- The sliding mask adds +1.0 uniformly inside the window: softmax-invariant.
- Out-of-window/causal positions are exactly zeroed by multiplying exp(score)
  with a 0/1 mask (scores are O(1) so exp never overflows).
- Scores are O(1), so softmax without max-subtraction is safe.

Implementation notes (v1, bf16):
- All activations/weights stream as bf16 (halves DMA + SBUF; PE rate is the
  same 1 cycle/row as fp32r, accumulation stays fp32 in PSUM).
- Big batched DMAs (one per weight tensor / hsT chunk) to amortize the shared
  HWDGE descriptor stage.
- Attention is software-pipelined: score matmuls + exp run 2 quads ahead of
  the AV/denominator matmuls so PE never waits on the Act engine.
- Wo is interleaved one query-group behind attention, spreading its PSUM
  drain + output DMA across the attention phase.
- PSUM->SBUF drains are spread across DVE/Act/Pool engines.
"""
import math
from contextlib import ExitStack

import numpy as np
import ml_dtypes

import concourse.tile as tile
from concourse import bacc, mybir
from concourse.bass_utils import run_bass_kernel_spmd
from concourse.masks import make_identity

dt = mybir.dt
BF16 = ml_dtypes.bfloat16

B, S, H = 2, 2048, 2048
NUM_HEADS, KV_HEADS, D = 16, 4, 128
WINDOW = 1024
GH = 4            # query heads per kv head (per core)
GD = GH * D       # 512: per-core slice of the hidden dim
SCALE = 1.0 / math.sqrt(D)
QB = 256          # query columns per attention group
NG = S // QB      # 8 query groups
KT = H // 128     # 16 contraction tiles for projections

_nc_cache = None


def _build_nc(depth=3):
    nc = bacc.Bacc()
    hsT = nc.declare_dram_parameter("hsT", [4, KT, 128, 512], dt.bfloat16, isOutput=False)
    wq = nc.declare_dram_parameter("wq", [KT, 128, GD], dt.bfloat16, isOutput=False)
    wkv = nc.declare_dram_parameter("wkv", [KT, 128, 2 * D], dt.bfloat16, isOutput=False)
    wo = nc.declare_dram_parameter("wo", [4, 128, H], dt.bfloat16, isOutput=False)
    masks = nc.declare_dram_parameter("masks", [128, 768], dt.bfloat16, isOutput=False)
    out = nc.declare_dram_parameter("out", [16, 4, 128, 512], dt.bfloat16, isOutput=True)

    with tile.TileContext(nc) as tc, ExitStack() as ctx:
        consts = ctx.enter_context(tc.tile_pool(name="consts", bufs=1))
        wpool = ctx.enter_context(tc.tile_pool(name="wpool", bufs=1))
        big = ctx.enter_context(tc.tile_pool(name="big", bufs=1))
        hstp = ctx.enter_context(tc.tile_pool(name="hstp", bufs=2))
        vtp = ctx.enter_context(tc.tile_pool(name="vtp", bufs=2))
        ptp = ctx.enter_context(tc.tile_pool(name="ptp", bufs=5))
        smalls = ctx.enter_context(tc.tile_pool(name="smalls", bufs=3))
        outp = ctx.enter_context(tc.tile_pool(name="outp", bufs=3))

        # constants
        ident32 = consts.tile([128, 128], dt.float32)
        make_identity(nc, ident32)
        ident = consts.tile([128, 128], dt.float32r)
        nc.vector.tensor_copy(ident, ident32)
        ones32 = consts.tile([128, 128], dt.float32)
        nc.vector.memset(ones32, 1.0)
        ones_bf = consts.tile([128, 128], dt.bfloat16)
        nc.vector.tensor_copy(ones_bf, ones32)
        mask_t = consts.tile([128, 768], dt.bfloat16)

        # weights (single big SBUF tiles, loaded with few big DMAs)
        wq_sb = wpool.tile([128, KT * GD], dt.bfloat16)      # 16KB/part
        wkv_sb = wpool.tile([128, KT * 2 * D], dt.bfloat16)  # 8KB/part
        wo_sb = wpool.tile([128, 4 * H], dt.bfloat16)        # 16KB/part

        # persistent activations (bf16)
        qT = [big.tile([128, S], dt.bfloat16, name=f"qT{h}") for h in range(GH)]
        kT = big.tile([128, S], dt.bfloat16)
        v = big.tile([128, S], dt.bfloat16)   # [key, d] layout per 128-block
        ohT = [big.tile([128, S], dt.bfloat16, name=f"ohT{h}") for h in range(GH)]

        wq_v = wq_sb.rearrange("p (t n) -> p t n", t=KT)
        wkv_v = wkv_sb.rearrange("p (t n) -> p t n", t=KT)

        # ---- Phase 1: projections (per 512-wide s-chunk) ----
        with tc.tile_pool(name="psA", bufs=8, space="PSUM") as psA:
            hst_tiles = []
            for ch in range(4):
                hst = hstp.tile([128, KT * 512], dt.bfloat16, tag="hst", name=f"hst{ch}")
                hst_tiles.append(hst)
            # chunk 0: quarter-granularity DMAs interleaved with weight quarters
            h0v = hst_tiles[0].rearrange("p (t n) -> p t n", t=KT)
            for sl in (slice(0, 1), slice(1, 2), slice(2, 4), slice(4, 8),
                       slice(8, 12), slice(12, 16)):
                nc.sync.dma_start(out=wq_v[:, sl], in_=wq[sl].rearrange("t p n -> p t n"))
                nc.sync.dma_start(out=wkv_v[:, sl], in_=wkv[sl].rearrange("t p n -> p t n"))
                nc.sync.dma_start(out=h0v[:, sl], in_=hsT[0, sl].rearrange("t p n -> p t n"))

            for ch in range(4):
                if ch + 1 < 4:
                    nxt = hst_tiles[ch + 1]
                    nc.sync.dma_start(
                        out=nxt.rearrange("p (t n) -> p t n", t=KT),
                        in_=hsT[ch + 1].rearrange("t p n -> p t n"))
                if ch == 0:
                    nc.sync.dma_start(out=mask_t, in_=masks[:, :])
                    nc.sync.dma_start(
                        out=wo_sb.rearrange("p (c n) -> p c n", c=4),
                        in_=wo[:].rearrange("c p n -> p c n"))
                hst = hst_tiles[ch]
                q_ps = [psA.tile([128, 512], dt.float32, tag="ps", name=f"qps{ch}_{h}")
                        for h in range(GH)]
                k_ps = psA.tile([128, 512], dt.float32, tag="ps")
                v_ps = psA.tile([128, 512], dt.float32, tag="ps")

                def vtrans(ch, vt):
                    # transpose V of a finished chunk, interleaved into the
                    # next chunk's matmul stream so PE never waits on it
                    for j in range(4):
                        tp = psA.tile([128, 128], dt.float32r, tag="ps",
                                      name=f"tp{ch}_{j}")
                        nc.tensor.transpose(tp, vt[:, j * 128:(j + 1) * 128], ident)
                        nc.scalar.copy(
                            v[:, (4 * ch + j) * 128:(4 * ch + j + 1) * 128], tp)

                for t in range(KT):
                    if ch > 0 and t == 4:
                        vtrans(ch - 1, prev_vt)
                    rhs = hst[:, t * 512:(t + 1) * 512]
                    st = (t == 0)
                    sp = (t == KT - 1)
                    for h in range(GH):
                        nc.tensor.matmul(
                            q_ps[h], lhsT=wq_sb[:, t * 512 + h * 128: t * 512 + (h + 1) * 128],
                            rhs=rhs, start=st, stop=sp)
                    nc.tensor.matmul(k_ps, lhsT=wkv_sb[:, t * 256: t * 256 + 128],
                                     rhs=rhs, start=st, stop=sp)
                    nc.tensor.matmul(v_ps, lhsT=wkv_sb[:, t * 256 + 128: t * 256 + 256],
                                     rhs=rhs, start=st, stop=sp)
                # drain PSUM on three engines in slot-rotation order
                cs = slice(ch * 512, (ch + 1) * 512)
                nc.vector.tensor_copy(qT[0][:, cs], q_ps[0])
                nc.scalar.copy(qT[1][:, cs], q_ps[1])
                nc.vector.tensor_copy(qT[2][:, cs], q_ps[2])
                nc.vector.tensor_copy(qT[3][:, cs], q_ps[3])
                nc.scalar.copy(kT[:, cs], k_ps)
                vt = vtp.tile([128, 512], dt.float32r, tag="vt")
                nc.scalar.copy(vt, v_ps)
                prev_vt = vt
            vtrans(3, prev_vt)

        # ---- Phase 2+3: banded attention (S^T[k,q] layout) + interleaved Wo ----
        mask_R = mask_t[:, 0:384]
        mask_L = mask_t[:, 384:768]
        with tc.tile_pool(name="psB", bufs=1, space="PSUM") as psB:
            pending = []
            fin_done = set()

            def drain(n):
                while len(pending) > n:
                    pending.pop(0)()

            wo_parts = []  # deferred per-(st,e) Wo emission closures

            def mk_wo(st, e, osb):
                def f():
                    wop = psB.tile([128, 512], dt.float32, tag="wop", bufs=2,
                                   name=f"wop{st}_{e}")
                    for ct in range(4):
                        nc.tensor.matmul(
                            wop, lhsT=ohT[ct][:, st * 128:(st + 1) * 128],
                            rhs=wo_sb[:, ct * 2048 + e * 512: ct * 2048 + (e + 1) * 512],
                            start=(ct == 0), stop=(ct == 3))
                    nc.scalar.copy(osb[:, e * 512:(e + 1) * 512], wop)
                    nc.sync.dma_start(
                        out=out[st, e], in_=osb[:, e * 512:(e + 1) * 512])
                return f

            def queue_wo(g):
                for st in (2 * g, 2 * g + 1):
                    osb = outp.tile([128, 2048], dt.bfloat16, tag="osb", name=f"osb{st}")
                    for e in range(4):
                        wo_parts.append(mk_wo(st, e, osb))

            for g in range(NG):
                if g >= 1:
                    queue_wo(g - 1)
                for h in range(GH):
                    kjs = list(range(max(0, 2 * g - 8), 2 * g + 2))
                    prs = [kjs[i:i + 2] for i in range(0, len(kjs), 2)]
                    nb = len(prs)
                    av = psB.tile([128, QB], dt.float32, tag="av", bufs=2,
                                  name=f"av{h}_{g}")
                    ptsum = None
                    prev_pt = None
                    for bi, pr in enumerate(prs):
                        kind = 'R' if bi == nb - 1 else ('L' if bi == 0 and g >= 4 else 'P')
                        # entries: (kj, col0, width, qoff); av order full-first
                        if kind == 'R':       # [o=+1 right-half | o=0 full]
                            ents = [(2 * g, 256, 256, 0), (2 * g + 1, 128, 128, 128)]
                            erg = slice(128, 512)
                            zrg = slice(0, 128)
                        elif kind == 'L':     # [o=-7 full | o=-8 left-half]
                            ents = [(2 * g - 7, 0, 256, 0), (2 * g - 8, 256, 128, 0)]
                            erg = slice(0, 384)
                            zrg = slice(384, 512)
                        else:
                            ents = [(pr[0], 0, 256, 0), (pr[1], 256, 256, 0)]
                            erg = slice(0, 512)
                            zrg = None
                        sps = psB.tile([128, 512], dt.float32, tag="sps", bufs=3,
                                       name=f"sps{h}_{g}_{bi}")
                        for kj, c0, w, qo in ents:
                            nc.tensor.matmul(
                                sps[:, c0:c0 + w],
                                lhsT=kT[:, kj * 128:(kj + 1) * 128],
                                rhs=qT[h][:, g * QB + qo:g * QB + qo + w],
                                start=True, stop=True)
                        pt = ptp.tile([128, 512], dt.bfloat16, tag="pt",
                                      name=f"pt{h}_{g}_{bi}")
                        if zrg is not None:
                            nc.gpsimd.memset(pt[:, zrg], 0.0)
                        nc.scalar.activation(
                            pt[:, erg], sps[:, erg],
                            mybir.ActivationFunctionType.Exp, scale=SCALE)
                        if kind == 'R':
                            nc.vector.tensor_mul(pt[:, erg], pt[:, erg], mask_R)
                        elif kind == 'L':
                            nc.vector.tensor_mul(pt[:, erg], pt[:, erg], mask_L)
                        if bi == 1:
                            ptsum = smalls.tile([128, 512], dt.bfloat16, tag="ptsum",
                                                name=f"ptsum{h}_{g}")

                        def mk_av(pt=pt, ents=ents, bi=bi, first=(bi == 0),
                                  last=(bi == nb - 1), av=av, ptsum=ptsum,
                                  prev_pt=prev_pt):
                            def f():
                                for i, (kj, c0, w, qo) in enumerate(ents):
                                    nc.tensor.matmul(
                                        av[:, qo:qo + w],
                                        lhsT=v[:, kj * 128:(kj + 1) * 128],
                                        rhs=pt[:, c0:c0 + w],
                                        start=(first and i == 0),
                                        stop=(last and i == len(ents) - 1))
                                # running pt-sum (softmax denominator); first
                                # add on the otherwise-idle Pool engine
                                if bi == 1:
                                    nc.gpsimd.tensor_add(ptsum, prev_pt, pt)
                                elif bi > 1:
                                    nc.vector.tensor_add(ptsum, ptsum, pt)
                            return f
                        pending.append(mk_av())
                        drain(depth)
                        prev_pt = pt

                    def mk_fin(h=h, g=g, av=av, ptsum=ptsum, pt=pt):
                        def f():
                            # fold the two kj-halves -> per-q key-sums [128, 256]
                            src = ptsum if ptsum is not None else pt
                            ptf = smalls.tile([128, QB], dt.bfloat16, tag="ptf",
                                              name=f"ptf{h}_{g}")
                            nc.vector.tensor_add(ptf, src[:, 0:QB], src[:, QB:2 * QB])
                            denb = psB.tile([128, QB], dt.float32, tag="den", bufs=1,
                                            name=f"den{h}_{g}")
                            nc.tensor.matmul(denb, lhsT=ones_bf, rhs=ptf,
                                             start=True, stop=True)
                            rcb = smalls.tile([128, QB], dt.float32r, tag="bcs",
                                              name=f"rcb{h}_{g}")
                            with nc.allow_low_precision(reason="f32r is full fp32 bits"):
                                nc.vector.reciprocal(rcb, denb)
                            nc.vector.tensor_mul(
                                ohT[h][:, g * QB:(g + 1) * QB], av, rcb)
                            fin_done.add((h, g))
                        return f
                    pending.append(mk_fin())
                    # interleave two Wo pieces of the previous group; their
                    # ohT inputs must have been written (fins emitted) first
                    if g >= 1:
                        while (3, g - 1) not in fin_done and pending:
                            pending.pop(0)()
                        for _ in range(2):
                            if wo_parts:
                                wo_parts.pop(0)()
            drain(0)
            queue_wo(NG - 1)
            while wo_parts:
                wo_parts.pop(0)()

    nc.compile()
    return nc


def _build_masks():
    kk = np.arange(128)[:, None]
    qq = np.arange(256)[None, :]
    cc = np.arange(128)[None, :]
    # mask_R covers pt cols [128:512] of an R quad: [o=+1 right half | o=0 full]
    r1 = (kk <= cc).astype(np.float32)             # o = +1 on q in [128:256)
    r0 = (kk <= qq).astype(np.float32)             # o = 0
    # mask_L covers pt cols [0:384] of an L quad: [o=-7 full | o=-8 left half]
    l1 = (kk + 128 >= qq).astype(np.float32)       # o = -7
    l0 = (kk >= cc).astype(np.float32)             # o = -8 on q in [0:128)
    return np.hstack([r1, r0, l1, l0]).astype(BF16)  # [128, 768]


def kernel(hidden_states, Wq, Wk, Wv, Wo):
    global _nc_cache
    if _nc_cache is None:
        _nc_cache = _build_nc()
    nc = _nc_cache

    masks = _build_masks()
    hsT = []
    for b in range(B):
        ht = np.ascontiguousarray(hidden_states[b].T)                 # [H, S]
        t4 = ht.reshape(KT, 128, 4, 512).transpose(2, 0, 1, 3)        # [ch, t, 128, 512]
        hsT.append(np.ascontiguousarray(t4).astype(BF16))
    in_maps = []
    for b in range(B):
        for gi in range(KV_HEADS):
            wkv = np.concatenate(
                [Wk[:, gi * D:(gi + 1) * D], Wv[:, gi * D:(gi + 1) * D]], axis=1)
            in_maps.append({
                "hsT": hsT[b],
                "wq": Wq[:, gi * GD:(gi + 1) * GD].reshape(KT, 128, GD).astype(BF16),
                "wkv": wkv.reshape(KT, 128, 2 * D).astype(BF16),
                "wo": Wo[gi * GD:(gi + 1) * GD, :].reshape(4, 128, H).astype(BF16),
                "masks": masks,
            })
    res = run_bass_kernel_spmd(nc, in_maps, list(range(8)))
    out = np.zeros((B, S, H), np.float32)
    for b in range(B):
        acc = None
        for gi in range(KV_HEADS):
            o = np.asarray(res.results[b * KV_HEADS + gi]["out"]).astype(np.float32)
            acc = o if acc is None else acc + o
        out[b] = acc.transpose(0, 2, 1, 3).reshape(S, H)              # [16,4,128,512] -> [S,H]
    return out


# revision 63
# speedup vs baseline: 2.0194x; 1.0127x over previous
"""GQA with sliding-window + ALiBi (reduces to banded causal attention) on 8 TRN2 cores.

Sharding: 8 cores = 2 batches x 4 kv-head groups. Each core computes, for its
(batch b, kv group gi): Q projection for its 4 query heads, K/V projection for
its 1 kv head, banded sliding-window attention (window 1024, causal), and a
partial row-parallel Wo matmul. Host sums the 4 partials per batch.

Math notes (exact reductions of the reference):
- ALiBi bias is -clip(j-i,0)*slope: zero on all causal positions, nonzero only
  where the causal mask kills the score -> drop it entirely.
- The sliding mask adds +1.0 uniformly inside the window: softmax-invariant.
- Out-of-window/causal positions are exactly zeroed by multiplying exp(score)
  with a 0/1 mask (scores are O(1) so exp never overflows).
- Scores are O(1), so softmax without max-subtraction is safe.

Implementation notes (v1, bf16):
- All activations/weights stream as bf16 (halves DMA + SBUF; PE rate is the
  same 1 cycle/row as fp32r, accumulation stays fp32 in PSUM).
- Big batched DMAs (one per weight tensor / hsT chunk) to amortize the shared
  HWDGE descriptor stage.
- Attention is software-pipelined: score matmuls + exp run 2 quads ahead of
  the AV/denominator matmuls so PE never waits on the Act engine.
- Wo is interleaved one query-group behind attention, spreading its PSUM
  drain + output DMA across the attention phase.
- PSUM->SBUF drains are spread across DVE/Act/Pool engines.
"""
import math
from contextlib import ExitStack

import numpy as np
import ml_dtypes

import concourse.tile as tile
from concourse import bacc, mybir
from concourse.bass_utils import run_bass_kernel_spmd
from concourse.masks import make_identity

dt = mybir.dt
BF16 = ml_dtypes.bfloat16

B, S, H = 2, 2048, 2048
NUM_HEADS, KV_HEADS, D = 16, 4, 128
WINDOW = 1024
GH = 4            # query heads per kv head (per core)
GD = GH * D       # 512: per-core slice of the hidden dim
SCALE = 1.0 / math.sqrt(D)
QB = 256          # query columns per attention group
NG = S // QB      # 8 query groups
KT = H // 128     # 16 contraction tiles for projections

_nc_cache = None


def _build_nc(depth=3):
    nc = bacc.Bacc()
    hsT = nc.declare_dram_parameter("hsT", [4, KT, 128, 512], dt.bfloat16, isOutput=False)
    wqkv = nc.declare_dram_parameter("wqkv", [KT, 128, GD + 2 * D], dt.bfloat16, isOutput=False)
    wo = nc.declare_dram_parameter("wo", [4, 128, H], dt.bfloat16, isOutput=False)
    masks = nc.declare_dram_parameter("masks", [128, 768], dt.bfloat16, isOutput=False)
    out = nc.declare_dram_parameter("out", [16, 4, 128, 512], dt.bfloat16, isOutput=True)

    with tile.TileContext(nc) as tc, ExitStack() as ctx:
        consts = ctx.enter_context(tc.tile_pool(name="consts", bufs=1))
        wpool = ctx.enter_context(tc.tile_pool(name="wpool", bufs=1))
        big = ctx.enter_context(tc.tile_pool(name="big", bufs=1))
        hstp = ctx.enter_context(tc.tile_pool(name="hstp", bufs=2))
        vtp = ctx.enter_context(tc.tile_pool(name="vtp", bufs=2))
        ptp = ctx.enter_context(tc.tile_pool(name="ptp", bufs=5))
        smalls = ctx.enter_context(tc.tile_pool(name="smalls", bufs=4))
        outp = ctx.enter_context(tc.tile_pool(name="outp", bufs=4))

        # constants
        ident32 = consts.tile([128, 128], dt.float32)
        make_identity(nc, ident32)
        ident = consts.tile([128, 128], dt.float32r)
        nc.vector.tensor_copy(ident, ident32)
        ones32 = consts.tile([128, 128], dt.float32)
        nc.vector.memset(ones32, 1.0)
        ones_bf = consts.tile([128, 128], dt.bfloat16)
        nc.vector.tensor_copy(ones_bf, ones32)
        mask_t = consts.tile([128, 768], dt.bfloat16)

        # weights (single big SBUF tiles, loaded with few big DMAs)
        wqkv_sb = wpool.tile([128, KT * (GD + 2 * D)], dt.bfloat16)  # 24KB/part
        wo_sb = wpool.tile([128, 4 * H], dt.bfloat16)        # 16KB/part

        # persistent activations (bf16)
        qT = [big.tile([128, S], dt.bfloat16, name=f"qT{h}") for h in range(GH)]
        kT = big.tile([128, S], dt.bfloat16)
        v = big.tile([128, S], dt.bfloat16)   # [key, d] layout per 128-block
        ohT = [big.tile([128, S], dt.bfloat16, name=f"ohT{h}") for h in range(GH)]

        wqkv_v = wqkv_sb.rearrange("p (t n) -> p t n", t=KT)

        # ---- Phase 2 machinery (shared between the psA and psB PSUM pools) --
        mask_R = mask_t[:, 0:384]
        mask_L = mask_t[:, 384:768]
        pending = []
        fin_done = set()

        def drain(n):
            while len(pending) > n:
                pending.pop(0)()

        wo_parts = []  # deferred per-(st,e) Wo emission closures

        def emit_head(g, h, pstile):
            kjs = list(range(max(0, 2 * g - 8), 2 * g + 2))
            prs = [kjs[i:i + 2] for i in range(0, len(kjs), 2)]
            nb = len(prs)
            av = pstile([128, QB], "av", 2, f"av{h}_{g}")
            ptsum = None
            prev_pt = None
            for bi, pr in enumerate(prs):
                kind = 'R' if bi == nb - 1 else ('L' if bi == 0 and g >= 4 else 'P')
                # entries: (kj, col0, width, qoff); av order full-first
                if kind == 'R':       # [o=+1 right-half | o=0 full]
                    ents = [(2 * g, 256, 256, 0), (2 * g + 1, 128, 128, 128)]
                    erg = slice(128, 512)
                    zrg = slice(0, 128)
                elif kind == 'L':     # [o=-7 full | o=-8 left-half]
                    ents = [(2 * g - 7, 0, 256, 0), (2 * g - 8, 256, 128, 0)]
                    erg = slice(0, 384)
                    zrg = slice(384, 512)
                else:
                    ents = [(pr[0], 0, 256, 0), (pr[1], 256, 256, 0)]
                    erg = slice(0, 512)
                    zrg = None
                sps = pstile([128, 512], "sps", 3, f"sps{h}_{g}_{bi}")
                for kj, c0, w, qo in ents:
                    nc.tensor.matmul(
                        sps[:, c0:c0 + w],
                        lhsT=kT[:, kj * 128:(kj + 1) * 128],
                        rhs=qT[h][:, g * QB + qo:g * QB + qo + w],
                        start=True, stop=True)
                pt = ptp.tile([128, 512], dt.bfloat16, tag="pt",
                              name=f"pt{h}_{g}_{bi}")
                if zrg is not None:
                    nc.gpsimd.memset(pt[:, zrg], 0.0)
                nc.scalar.activation(
                    pt[:, erg], sps[:, erg],
                    mybir.ActivationFunctionType.Exp, scale=SCALE)
                if kind == 'R':
                    nc.vector.tensor_mul(pt[:, erg], pt[:, erg], mask_R)
                elif kind == 'L':
                    nc.vector.tensor_mul(pt[:, erg], pt[:, erg], mask_L)
                if bi == 1:
                    ptsum = smalls.tile([128, 512], dt.bfloat16, tag="ptsum",
                                        name=f"ptsum{h}_{g}")

                def mk_av(pt=pt, ents=ents, bi=bi, first=(bi == 0),
                          last=(bi == nb - 1), av=av, ptsum=ptsum,
                          prev_pt=prev_pt):
                    def f():
                        for i, (kj, c0, w, qo) in enumerate(ents):
                            nc.tensor.matmul(
                                av[:, qo:qo + w],
                                lhsT=v[:, kj * 128:(kj + 1) * 128],
                                rhs=pt[:, c0:c0 + w],
                                start=(first and i == 0),
                                stop=(last and i == len(ents) - 1))
                        # running pt-sum (softmax denominator); first
                        # add on the otherwise-idle Pool engine
                        if bi == 1:
                            nc.gpsimd.tensor_add(ptsum, prev_pt, pt)
                        elif bi > 1:
                            nc.vector.tensor_add(ptsum, ptsum, pt)
                    return f
                pending.append(mk_av())
                drain(depth)
                prev_pt = pt

            def mk_fin(h=h, g=g, av=av, ptsum=ptsum, pt=pt, pstile=pstile):
                def f():
                    # fold the two kj-halves -> per-q key-sums [128, 256]
                    src = ptsum if ptsum is not None else pt
                    ptf = smalls.tile([128, QB], dt.bfloat16, tag="ptf",
                                      name=f"ptf{h}_{g}")
                    nc.vector.tensor_add(ptf, src[:, 0:QB], src[:, QB:2 * QB])
                    denb = pstile([128, QB], "den", 1, f"den{h}_{g}")
                    nc.tensor.matmul(denb, lhsT=ones_bf, rhs=ptf,
                                     start=True, stop=True)
                    rcb = smalls.tile([128, QB], dt.float32r, tag="bcs",
                                      name=f"rcb{h}_{g}")
                    with nc.allow_low_precision(reason="f32r is full fp32 bits"):
                        nc.vector.reciprocal(rcb, denb)
                    nc.vector.tensor_mul(
                        ohT[h][:, g * QB:(g + 1) * QB], av, rcb)
                    fin_done.add((h, g))
                return f
            pending.append(mk_fin())

        # ---- Phase 1: projections (per 512-wide s-chunk) ----
        with tc.tile_pool(name="psA", bufs=8, space="PSUM") as psA:
            hst_tiles = []
            for ch in range(4):
                hst = hstp.tile([128, KT * 512], dt.bfloat16, tag="hst", name=f"hst{ch}")
                hst_tiles.append(hst)
            # chunk 0: quarter-granularity DMAs interleaved with weight quarters
            h0v = hst_tiles[0].rearrange("p (t n) -> p t n", t=KT)
            for sl in (slice(0, 1), slice(1, 2), slice(2, 4), slice(4, 8),
                       slice(8, 12), slice(12, 16)):
                nc.sync.dma_start(out=wqkv_v[:, sl], in_=wqkv[sl].rearrange("t p n -> p t n"))
                nc.sync.dma_start(out=h0v[:, sl], in_=hsT[0, sl].rearrange("t p n -> p t n"))

            for ch in range(4):
                if ch + 1 < 4:
                    nxt = hst_tiles[ch + 1]
                    nc.sync.dma_start(
                        out=nxt.rearrange("p (t n) -> p t n", t=KT),
                        in_=hsT[ch + 1].rearrange("t p n -> p t n"))
                if ch == 0:
                    nc.sync.dma_start(out=mask_t, in_=masks[:, :])
                    nc.sync.dma_start(
                        out=wo_sb.rearrange("p (c n) -> p c n", c=4),
                        in_=wo[:].rearrange("c p n -> p c n"))
                hst = hst_tiles[ch]
                q_ps = [psA.tile([128, 512], dt.float32, tag="ps", name=f"qps{ch}_{h}")
                        for h in range(GH)]
                k_ps = psA.tile([128, 512], dt.float32, tag="ps")
                v_ps = psA.tile([128, 512], dt.float32, tag="ps")

                def vtrans(ch, vt):
                    # transpose V of a finished chunk, interleaved into the
                    # next chunk's matmul stream so PE never waits on it
                    for j in range(4):
                        tp = psA.tile([128, 128], dt.float32r, tag="ps",
                                      name=f"tp{ch}_{j}")
                        nc.tensor.transpose(tp, vt[:, j * 128:(j + 1) * 128], ident)
                        nc.scalar.copy(
                            v[:, (4 * ch + j) * 128:(4 * ch + j + 1) * 128], tp)

                for t in range(KT):
                    if ch > 0 and t == 4:
                        vtrans(ch - 1, prev_vt)
                    rhs = hst[:, t * 512:(t + 1) * 512]
                    st = (t == 0)
                    sp = (t == KT - 1)
                    for h in range(GH):
                        nc.tensor.matmul(
                            q_ps[h], lhsT=wqkv_sb[:, t * 768 + h * 128: t * 768 + (h + 1) * 128],
                            rhs=rhs, start=st, stop=sp)
                    nc.tensor.matmul(k_ps, lhsT=wqkv_sb[:, t * 768 + 512: t * 768 + 640],
                                     rhs=rhs, start=st, stop=sp)
                    nc.tensor.matmul(v_ps, lhsT=wqkv_sb[:, t * 768 + 640: t * 768 + 768],
                                     rhs=rhs, start=st, stop=sp)
                # drain PSUM on three engines in slot-rotation order
                cs = slice(ch * 512, (ch + 1) * 512)
                nc.vector.tensor_copy(qT[0][:, cs], q_ps[0])
                nc.scalar.copy(qT[1][:, cs], q_ps[1])
                nc.vector.tensor_copy(qT[2][:, cs], q_ps[2])
                nc.vector.tensor_copy(qT[3][:, cs], q_ps[3])
                nc.scalar.copy(kT[:, cs], k_ps)
                vt = vtp.tile([128, 512], dt.float32r, tag="vt")
                nc.scalar.copy(vt, v_ps)
                prev_vt = vt
            vtrans(3, prev_vt)

        # ---- Phase 2+3: banded attention (S^T[k,q] layout) + interleaved Wo ----
        with tc.tile_pool(name="psB", bufs=1, space="PSUM") as psB:
            def psb_tile(shape, tag, bufs, name):
                return psB.tile(shape, dt.float32, tag=tag, bufs=bufs, name=name)

            def mk_wo(st, e, osb):
                def f():
                    wop = psB.tile([128, 512], dt.float32, tag="wop", bufs=2,
                                   name=f"wop{st}_{e}")
                    for ct in range(4):
                        nc.tensor.matmul(
                            wop, lhsT=ohT[ct][:, st * 128:(st + 1) * 128],
                            rhs=wo_sb[:, ct * 2048 + e * 512: ct * 2048 + (e + 1) * 512],
                            start=(ct == 0), stop=(ct == 3))
                    nc.scalar.copy(osb[:, e * 512:(e + 1) * 512], wop)
                    nc.sync.dma_start(
                        out=out[st, e], in_=osb[:, e * 512:(e + 1) * 512])
                return f

            def queue_wo(g):
                for st in (2 * g, 2 * g + 1):
                    osb = outp.tile([128, 2048], dt.bfloat16, tag="osb", name=f"osb{st}")
                    for e in range(4):
                        wo_parts.append(mk_wo(st, e, osb))

            for g in range(NG):
                if g >= 1:
                    queue_wo(g - 1)
                npop = 2
                for h in range(GH):
                    emit_head(g, h, psb_tile)
                    # interleave Wo pieces of earlier groups; their ohT
                    # inputs must have been written (fins emitted) first
                    if g >= 1:
                        while (3, g - 1) not in fin_done and pending:
                            pending.pop(0)()
                        for _ in range(npop):
                            if wo_parts:
                                wo_parts.pop(0)()
            drain(0)
            queue_wo(NG - 1)
            while wo_parts:
                wo_parts.pop(0)()

    nc.compile()
    return nc


def _build_masks():
    kk = np.arange(128)[:, None]
    qq = np.arange(256)[None, :]
    cc = np.arange(128)[None, :]
    # mask_R covers pt cols [128:512] of an R quad: [o=+1 right half | o=0 full]
    r1 = (kk <= cc).astype(np.float32)             # o = +1 on q in [128:256)
    r0 = (kk <= qq).astype(np.float32)             # o = 0
    # mask_L covers pt cols [0:384] of an L quad: [o=-7 full | o=-8 left half]
    l1 = (kk + 128 >= qq).astype(np.float32)       # o = -7
    l0 = (kk >= cc).astype(np.float32)             # o = -8 on q in [0:128)
    return np.hstack([r1, r0, l1, l0]).astype(BF16)  # [128, 768]


def kernel(hidden_states, Wq, Wk, Wv, Wo):
    global _nc_cache
    if _nc_cache is None:
        _nc_cache = _build_nc()
    nc = _nc_cache

    masks = _build_masks()
    hsT = []
    for b in range(B):
        ht = np.ascontiguousarray(hidden_states[b].T)                 # [H, S]
        t4 = ht.reshape(KT, 128, 4, 512).transpose(2, 0, 1, 3)        # [ch, t, 128, 512]
        hsT.append(np.ascontiguousarray(t4).astype(BF16))
    in_maps = []
    for b in range(B):
        for gi in range(KV_HEADS):
            wqkv = np.concatenate(
                [Wq[:, gi * GD:(gi + 1) * GD], Wk[:, gi * D:(gi + 1) * D],
                 Wv[:, gi * D:(gi + 1) * D]], axis=1)
            in_maps.append({
                "hsT": hsT[b],
                "wqkv": wqkv.reshape(KT, 128, GD + 2 * D).astype(BF16),
                "wo": Wo[gi * GD:(gi + 1) * GD, :].reshape(4, 128, H).astype(BF16),
                "masks": masks,
            })
    res = run_bass_kernel_spmd(nc, in_maps, list(range(8)))
    out = np.zeros((B, S, H), np.float32)
    for b in range(B):
        acc = None
        for gi in range(KV_HEADS):
            o = np.asarray(res.results[b * KV_HEADS + gi]["out"]).astype(np.float32)
            acc = o if acc is None else acc + o
        out[b] = acc.transpose(0, 2, 1, 3).reshape(S, H)              # [16,4,128,512] -> [S,H]
    return out


# revision 70
# speedup vs baseline: 2.0334x; 1.0069x over previous
"""GQA with sliding-window + ALiBi (reduces to banded causal attention) on 8 TRN2 cores.

Sharding: 8 cores = 2 batches x 4 kv-head groups. Each core computes, for its
(batch b, kv group gi): Q projection for its 4 query heads, K/V projection for
its 1 kv head, banded sliding-window attention (window 1024, causal), and a
partial row-parallel Wo matmul. Host sums the 4 partials per batch.

Math notes (exact reductions of the reference):
- ALiBi bias is -clip(j-i,0)*slope: zero on all causal positions, nonzero only
  where the causal mask kills the score -> drop it entirely.
- The sliding mask adds +1.0 uniformly inside the window: softmax-invariant.
- Out-of-window/causal positions are exactly zeroed by multiplying exp(score)
  with a 0/1 mask (scores are O(1) so exp never overflows).
- Scores are O(1), so softmax without max-subtraction is safe.

Implementation notes (v1, bf16):
- All activations/weights stream as bf16 (halves DMA + SBUF; PE rate is the
  same 1 cycle/row as fp32r, accumulation stays fp32 in PSUM).
- Big batched DMAs (one per weight tensor / hsT chunk) to amortize the shared
  HWDGE descriptor stage.
- Attention is software-pipelined: score matmuls + exp run 3 quads ahead of
  the AV/denominator matmuls so PE never waits on the Act engine.
- Wo is interleaved one query-group behind attention, spreading its PSUM
  drain + output DMA across the attention phase.
- PSUM->SBUF drains are spread across DVE/Act/Pool engines.
"""
import math
from contextlib import ExitStack

import numpy as np
import ml_dtypes

import concourse.tile as tile
from concourse import bacc, mybir
from concourse.bass_utils import run_bass_kernel_spmd
from concourse.masks import make_identity

dt = mybir.dt
BF16 = ml_dtypes.bfloat16

B, S, H = 2, 2048, 2048
NUM_HEADS, KV_HEADS, D = 16, 4, 128
WINDOW = 1024
GH = 4            # query heads per kv head (per core)
GD = GH * D       # 512: per-core slice of the hidden dim
SCALE = 1.0 / math.sqrt(D)
QB = 256          # query columns per attention group
NG = S // QB      # 8 query groups
KT = H // 128     # 16 contraction tiles for projections

_nc_cache = None


def _build_nc(depth=3):
    nc = bacc.Bacc()
    hsT = nc.declare_dram_parameter("hsT", [4, KT, 128, 512], dt.bfloat16, isOutput=False)
    wqkv = nc.declare_dram_parameter("wqkv", [KT, 128, GD + 2 * D], dt.bfloat16, isOutput=False)
    wo = nc.declare_dram_parameter("wo", [4, 128, H], dt.bfloat16, isOutput=False)
    masks = nc.declare_dram_parameter("masks", [128, 768], dt.bfloat16, isOutput=False)
    out = nc.declare_dram_parameter("out", [16, 4, 128, 512], dt.bfloat16, isOutput=True)

    with tile.TileContext(nc) as tc, ExitStack() as ctx:
        consts = ctx.enter_context(tc.tile_pool(name="consts", bufs=1))
        wpool = ctx.enter_context(tc.tile_pool(name="wpool", bufs=1))
        big = ctx.enter_context(tc.tile_pool(name="big", bufs=1))
        hstp = ctx.enter_context(tc.tile_pool(name="hstp", bufs=2))
        vtp = ctx.enter_context(tc.tile_pool(name="vtp", bufs=2))
        ptp = ctx.enter_context(tc.tile_pool(name="ptp", bufs=5))
        smalls = ctx.enter_context(tc.tile_pool(name="smalls", bufs=4))
        outp = ctx.enter_context(tc.tile_pool(name="outp", bufs=4))

        # constants
        ident32 = consts.tile([128, 128], dt.float32)
        make_identity(nc, ident32)
        ident = consts.tile([128, 128], dt.float32r)
        nc.vector.tensor_copy(ident, ident32)
        ones32 = consts.tile([128, 128], dt.float32)
        nc.vector.memset(ones32, 1.0)
        ones_bf = consts.tile([128, 128], dt.bfloat16)
        nc.vector.tensor_copy(ones_bf, ones32)
        mask_t = consts.tile([128, 768], dt.bfloat16)

        # weights (single big SBUF tiles, loaded with few big DMAs)
        wqkv_sb = wpool.tile([128, KT * (GD + 2 * D)], dt.bfloat16)  # 24KB/part
        wo_sb = wpool.tile([128, 4 * H], dt.bfloat16)        # 16KB/part

        # persistent activations (bf16)
        qT = [big.tile([128, S], dt.bfloat16, name=f"qT{h}") for h in range(GH)]
        kT = big.tile([128, S], dt.bfloat16)
        v = big.tile([128, S], dt.bfloat16)   # [key, d] layout per 128-block
        ohT = [big.tile([128, S], dt.bfloat16, name=f"ohT{h}") for h in range(GH)]

        wqkv_v = wqkv_sb.rearrange("p (t n) -> p t n", t=KT)

        # ---- Phase 2 machinery (shared between the psA and psB PSUM pools) --
        mask_R = mask_t[:, 0:384]
        mask_L = mask_t[:, 384:768]
        pending = []
        fin_done = set()

        def drain(n):
            while len(pending) > n:
                pending.pop(0)()

        wo_parts = []  # deferred per-(st,e) Wo emission closures

        def emit_head(g, h, pstile):
            kjs = list(range(max(0, 2 * g - 8), 2 * g + 2))
            prs = [kjs[i:i + 2] for i in range(0, len(kjs), 2)]
            nb = len(prs)
            av = pstile([128, QB], "av", 2, f"av{h}_{g}")
            ptsum = None
            prev_pt = None
            for bi, pr in enumerate(prs):
                kind = 'R' if bi == nb - 1 else ('L' if bi == 0 and g >= 4 else 'P')
                # entries: (kj, col0, width, qoff); av order full-first
                if kind == 'R':       # [o=+1 right-half | o=0 full]
                    ents = [(2 * g, 256, 256, 0), (2 * g + 1, 128, 128, 128)]
                    erg = slice(128, 512)
                    zrg = slice(0, 128)
                elif kind == 'L':     # [o=-7 full | o=-8 left-half]
                    ents = [(2 * g - 7, 0, 256, 0), (2 * g - 8, 256, 128, 0)]
                    erg = slice(0, 384)
                    zrg = slice(384, 512)
                else:
                    ents = [(pr[0], 0, 256, 0), (pr[1], 256, 256, 0)]
                    erg = slice(0, 512)
                    zrg = None
                sps = pstile([128, 512], "sps", 3, f"sps{h}_{g}_{bi}")
                for kj, c0, w, qo in ents:
                    nc.tensor.matmul(
                        sps[:, c0:c0 + w],
                        lhsT=kT[:, kj * 128:(kj + 1) * 128],
                        rhs=qT[h][:, g * QB + qo:g * QB + qo + w],
                        start=True, stop=True)
                pt = ptp.tile([128, 512], dt.bfloat16, tag="pt",
                              name=f"pt{h}_{g}_{bi}")
                if zrg is not None:
                    nc.gpsimd.memset(pt[:, zrg], 0.0)
                nc.scalar.activation(
                    pt[:, erg], sps[:, erg],
                    mybir.ActivationFunctionType.Exp, scale=SCALE)
                if kind == 'R':
                    nc.vector.tensor_mul(pt[:, erg], pt[:, erg], mask_R)
                elif kind == 'L':
                    nc.vector.tensor_mul(pt[:, erg], pt[:, erg], mask_L)
                if bi == 1:
                    ptsum = smalls.tile([128, 512], dt.bfloat16, tag="ptsum",
                                        name=f"ptsum{h}_{g}")

                def mk_av(pt=pt, ents=ents, bi=bi, first=(bi == 0),
                          last=(bi == nb - 1), av=av, ptsum=ptsum,
                          prev_pt=prev_pt):
                    def f():
                        for i, (kj, c0, w, qo) in enumerate(ents):
                            nc.tensor.matmul(
                                av[:, qo:qo + w],
                                lhsT=v[:, kj * 128:(kj + 1) * 128],
                                rhs=pt[:, c0:c0 + w],
                                start=(first and i == 0),
                                stop=(last and i == len(ents) - 1))
                        # running pt-sum (softmax denominator); first
                        # add on the otherwise-idle Pool engine
                        if bi == 1:
                            nc.gpsimd.tensor_add(ptsum, prev_pt, pt)
                        elif bi > 1:
                            nc.vector.tensor_add(ptsum, ptsum, pt)
                    return f
                pending.append(mk_av())
                drain(depth)
                prev_pt = pt

            def mk_fin(h=h, g=g, av=av, ptsum=ptsum, pt=pt, pstile=pstile):
                def f():
                    # fold the two kj-halves -> per-q key-sums [128, 256]
                    src = ptsum if ptsum is not None else pt
                    ptf = smalls.tile([128, QB], dt.bfloat16, tag="ptf",
                                      name=f"ptf{h}_{g}")
                    nc.vector.tensor_add(ptf, src[:, 0:QB], src[:, QB:2 * QB])
                    denb = pstile([128, QB], "den", 1, f"den{h}_{g}")
                    nc.tensor.matmul(denb, lhsT=ones_bf, rhs=ptf,
                                     start=True, stop=True)
                    rcb = smalls.tile([128, QB], dt.float32r, tag="bcs",
                                      name=f"rcb{h}_{g}")
                    with nc.allow_low_precision(reason="f32r is full fp32 bits"):
                        nc.vector.reciprocal(rcb, denb)
                    nc.vector.tensor_mul(
                        ohT[h][:, g * QB:(g + 1) * QB], av, rcb)
                    fin_done.add((h, g))
                return f
            pending.append(mk_fin())

        # ---- Phase 1: projections (per 512-wide s-chunk) ----
        with tc.tile_pool(name="psA", bufs=8, space="PSUM") as psA:
            hst_tiles = []
            for ch in range(4):
                hst = hstp.tile([128, KT * 512], dt.bfloat16, tag="hst", name=f"hst{ch}")
                hst_tiles.append(hst)
            # chunk 0: quarter-granularity DMAs interleaved with weight quarters
            h0v = hst_tiles[0].rearrange("p (t n) -> p t n", t=KT)
            for sl in (slice(0, 1), slice(1, 2), slice(2, 4), slice(4, 7),
                       slice(7, 11), slice(11, 16)):
                nc.sync.dma_start(out=wqkv_v[:, sl], in_=wqkv[sl].rearrange("t p n -> p t n"))
                nc.sync.dma_start(out=h0v[:, sl], in_=hsT[0, sl].rearrange("t p n -> p t n"))

            for ch in range(4):
                if ch + 1 < 4:
                    nxt = hst_tiles[ch + 1]
                    nc.sync.dma_start(
                        out=nxt.rearrange("p (t n) -> p t n", t=KT),
                        in_=hsT[ch + 1].rearrange("t p n -> p t n"))
                if ch == 0:
                    nc.sync.dma_start(out=mask_t, in_=masks[:, :])
                    nc.sync.dma_start(
                        out=wo_sb.rearrange("p (c n) -> p c n", c=4),
                        in_=wo[:].rearrange("c p n -> p c n"))
                hst = hst_tiles[ch]
                q_ps = [psA.tile([128, 512], dt.float32, tag="ps", name=f"qps{ch}_{h}")
                        for h in range(GH)]
                k_ps = psA.tile([128, 512], dt.float32, tag="ps")
                v_ps = psA.tile([128, 512], dt.float32, tag="ps")

                def vtrans(ch, vt, pool=None):
                    # transpose V of a finished chunk, interleaved into the
                    # next chunk's matmul stream so PE never waits on it
                    for j in range(4):
                        if pool is None:
                            tp = psA.tile([128, 128], dt.float32r, tag="ps",
                                          name=f"tp{ch}_{j}")
                        else:
                            tp = pool.tile([128, 128], dt.float32r, tag="sps",
                                           bufs=3, name=f"tp{ch}_{j}")
                        nc.tensor.transpose(tp, vt[:, j * 128:(j + 1) * 128], ident)
                        nc.scalar.copy(
                            v[:, (4 * ch + j) * 128:(4 * ch + j + 1) * 128], tp)

                for t in range(KT):
                    if ch > 0 and t == 4:
                        vtrans(ch - 1, prev_vt)
                    rhs = hst[:, t * 512:(t + 1) * 512]
                    st = (t == 0)
                    sp = (t == KT - 1)
                    for h in range(GH):
                        nc.tensor.matmul(
                            q_ps[h], lhsT=wqkv_sb[:, t * 768 + h * 128: t * 768 + (h + 1) * 128],
                            rhs=rhs, start=st, stop=sp)
                    nc.tensor.matmul(k_ps, lhsT=wqkv_sb[:, t * 768 + 512: t * 768 + 640],
                                     rhs=rhs, start=st, stop=sp)
                    nc.tensor.matmul(v_ps, lhsT=wqkv_sb[:, t * 768 + 640: t * 768 + 768],
                                     rhs=rhs, start=st, stop=sp)
                # drain PSUM on three engines in slot-rotation order
                cs = slice(ch * 512, (ch + 1) * 512)
                nc.vector.tensor_copy(qT[0][:, cs], q_ps[0])
                nc.scalar.copy(qT[1][:, cs], q_ps[1])
                nc.vector.tensor_copy(qT[2][:, cs], q_ps[2])
                nc.vector.tensor_copy(qT[3][:, cs], q_ps[3])
                nc.scalar.copy(kT[:, cs], k_ps)
                vt = vtp.tile([128, 512], dt.float32r, tag="vt")
                nc.scalar.copy(vt, v_ps)
                prev_vt = vt

        # ---- Phase 2+3: banded attention (S^T[k,q] layout) + interleaved Wo ----
        with tc.tile_pool(name="psB", bufs=1, space="PSUM") as psB:
            def psb_tile(shape, tag, bufs, name):
                return psB.tile(shape, dt.float32, tag=tag, bufs=bufs, name=name)

            vtrans(3, prev_vt, pool=psB)

            def mk_wo(st, e, osb):
                def f():
                    wop = psB.tile([128, 512], dt.float32, tag="wop", bufs=2,
                                   name=f"wop{st}_{e}")
                    for ct in range(4):
                        nc.tensor.matmul(
                            wop, lhsT=ohT[ct][:, st * 128:(st + 1) * 128],
                            rhs=wo_sb[:, ct * 2048 + e * 512: ct * 2048 + (e + 1) * 512],
                            start=(ct == 0), stop=(ct == 3))
                    nc.scalar.copy(osb[:, e * 512:(e + 1) * 512], wop)
                    nc.sync.dma_start(
                        out=out[st, e], in_=osb[:, e * 512:(e + 1) * 512])
                return f

            def queue_wo(g):
                for st in (2 * g, 2 * g + 1):
                    osb = outp.tile([128, 2048], dt.bfloat16, tag="osb", name=f"osb{st}")
                    for e in range(4):
                        wo_parts.append(mk_wo(st, e, osb))

            for g in range(NG):
                if g >= 1:
                    queue_wo(g - 1)
                npop = 2
                for h in range(GH):
                    emit_head(g, h, psb_tile)
                    # interleave Wo pieces of earlier groups; their ohT
                    # inputs must have been written (fins emitted) first
                    if g >= 1:
                        while (3, g - 1) not in fin_done and pending:
                            pending.pop(0)()
                        for _ in range(npop):
                            if wo_parts:
                                wo_parts.pop(0)()
            drain(0)
            queue_wo(NG - 1)
            while wo_parts:
                wo_parts.pop(0)()

    nc.compile()
    return nc


def _build_masks():
    kk = np.arange(128)[:, None]
    qq = np.arange(256)[None, :]
    cc = np.arange(128)[None, :]
    # mask_R covers pt cols [128:512] of an R quad: [o=+1 right half | o=0 full]
    r1 = (kk <= cc).astype(np.float32)             # o = +1 on q in [128:256)
    r0 = (kk <= qq).astype(np.float32)             # o = 0
    # mask_L covers pt cols [0:384] of an L quad: [o=-7 full | o=-8 left half]
    l1 = (kk + 128 >= qq).astype(np.float32)       # o = -7
    l0 = (kk >= cc).astype(np.float32)             # o = -8 on q in [0:128)
    return np.hstack([r1, r0, l1, l0]).astype(BF16)  # [128, 768]


def kernel(hidden_states, Wq, Wk, Wv, Wo):
    global _nc_cache
    if _nc_cache is None:
        _nc_cache = _build_nc()
    nc = _nc_cache

    masks = _build_masks()
    hsT = []
    for b in range(B):
        ht = np.ascontiguousarray(hidden_states[b].T)                 # [H, S]
        t4 = ht.reshape(KT, 128, 4, 512).transpose(2, 0, 1, 3)        # [ch, t, 128, 512]
        hsT.append(np.ascontiguousarray(t4).astype(BF16))
    in_maps = []
    for b in range(B):
        for gi in range(KV_HEADS):
            wqkv = np.concatenate(
                [Wq[:, gi * GD:(gi + 1) * GD], Wk[:, gi * D:(gi + 1) * D],
                 Wv[:, gi * D:(gi + 1) * D]], axis=1)
            in_maps.append({
                "hsT": hsT[b],
                "wqkv": wqkv.reshape(KT, 128, GD + 2 * D).astype(BF16),
                "wo": Wo[gi * GD:(gi + 1) * GD, :].reshape(4, 128, H).astype(BF16),
                "masks": masks,
            })
    res = run_bass_kernel_spmd(nc, in_maps, list(range(8)))
    out = np.zeros((B, S, H), np.float32)
    for b in range(B):
        acc = None
        for gi in range(KV_HEADS):
            o = np.asarray(res.results[b * KV_HEADS + gi]["out"]).astype(np.float32)
            acc = o if acc is None else acc + o
        out[b] = acc.transpose(0, 2, 1, 3).reshape(S, H)              # [16,4,128,512] -> [S,H]
    return out


# revision 74
# speedup vs baseline: 2.0374x; 1.0020x over previous
"""GQA with sliding-window + ALiBi (reduces to banded causal attention) on 8 TRN2 cores.

Sharding: 8 cores = 2 batches x 4 kv-head groups. Each core computes, for its
(batch b, kv group gi): Q projection for its 4 query heads, K/V projection for
its 1 kv head, banded sliding-window attention (window 1024, causal), and a
partial row-parallel Wo matmul. Host sums the 4 partials per batch.

Math notes (exact reductions of the reference):
- ALiBi bias is -clip(j-i,0)*slope: zero on all causal positions, nonzero only
  where the causal mask kills the score -> drop it entirely.
- The sliding mask adds +1.0 uniformly inside the window: softmax-invariant.
- Out-of-window/causal positions are exactly zeroed by multiplying exp(score)
  with a 0/1 mask (scores are O(1) so exp never overflows).
- Scores are O(1), so softmax without max-subtraction is safe.

Implementation notes (v1, bf16):
- All activations/weights stream as bf16 (halves DMA + SBUF; PE rate is the
  same 1 cycle/row as fp32r, accumulation stays fp32 in PSUM).
- Big batched DMAs (one per weight tensor / hsT chunk) to amortize the shared
  HWDGE descriptor stage.
- Attention is software-pipelined: score matmuls + exp run 3 quads ahead of
  the AV/denominator matmuls so PE never waits on the Act engine.
- Wo is interleaved one query-group behind attention, spreading its PSUM
  drain + output DMA across the attention phase.
- PSUM->SBUF drains are spread across DVE/Act/Pool engines.
"""
import math
from contextlib import ExitStack

import numpy as np
import ml_dtypes

import concourse.tile as tile
from concourse import bacc, mybir
from concourse.bass_utils import run_bass_kernel_spmd
from concourse.masks import make_identity

dt = mybir.dt
BF16 = ml_dtypes.bfloat16

B, S, H = 2, 2048, 2048
NUM_HEADS, KV_HEADS, D = 16, 4, 128
WINDOW = 1024
GH = 4            # query heads per kv head (per core)
GD = GH * D       # 512: per-core slice of the hidden dim
SCALE = 1.0 / math.sqrt(D)
QB = 256          # query columns per attention group
NG = S // QB      # 8 query groups
KT = H // 128     # 16 contraction tiles for projections

_nc_cache = None


def _build_nc(depth=3):
    nc = bacc.Bacc()
    hsT = nc.declare_dram_parameter("hsT", [4, KT, 128, 512], dt.bfloat16, isOutput=False)
    wqkv = nc.declare_dram_parameter("wqkv", [KT, 128, GD + 2 * D], dt.bfloat16, isOutput=False)
    wo = nc.declare_dram_parameter("wo", [4, 128, H], dt.bfloat16, isOutput=False)
    masks = nc.declare_dram_parameter("masks", [128, 768], dt.bfloat16, isOutput=False)
    out = nc.declare_dram_parameter("out", [16, 4, 128, 512], dt.bfloat16, isOutput=True)

    with tile.TileContext(nc) as tc, ExitStack() as ctx:
        consts = ctx.enter_context(tc.tile_pool(name="consts", bufs=1))
        wpool = ctx.enter_context(tc.tile_pool(name="wpool", bufs=1))
        big = ctx.enter_context(tc.tile_pool(name="big", bufs=1))
        hstp = ctx.enter_context(tc.tile_pool(name="hstp", bufs=2))
        vtp = ctx.enter_context(tc.tile_pool(name="vtp", bufs=2))
        ptp = ctx.enter_context(tc.tile_pool(name="ptp", bufs=6))
        smalls = ctx.enter_context(tc.tile_pool(name="smalls", bufs=5))
        outp = ctx.enter_context(tc.tile_pool(name="outp", bufs=4))

        # constants
        ident32 = consts.tile([128, 128], dt.float32)
        make_identity(nc, ident32)
        ident = consts.tile([128, 128], dt.float32r)
        nc.vector.tensor_copy(ident, ident32)
        ones32 = consts.tile([128, 128], dt.float32)
        nc.vector.memset(ones32, 1.0)
        ones_bf = consts.tile([128, 128], dt.bfloat16)
        nc.vector.tensor_copy(ones_bf, ones32)
        mask_t = consts.tile([128, 768], dt.bfloat16)

        # weights (single big SBUF tiles, loaded with few big DMAs)
        wqkv_sb = wpool.tile([128, KT * (GD + 2 * D)], dt.bfloat16)  # 24KB/part
        wo_sb = wpool.tile([128, 4 * H], dt.bfloat16)        # 16KB/part

        # persistent activations (bf16)
        qT = [big.tile([128, S], dt.bfloat16, name=f"qT{h}") for h in range(GH)]
        kT = big.tile([128, S], dt.bfloat16)
        v = big.tile([128, S], dt.bfloat16)   # [key, d] layout per 128-block
        ohT = [big.tile([128, S], dt.bfloat16, name=f"ohT{h}") for h in range(GH)]

        wqkv_v = wqkv_sb.rearrange("p (t n) -> p t n", t=KT)

        # ---- Phase 2 machinery (shared between the psA and psB PSUM pools) --
        mask_R = mask_t[:, 0:384]
        mask_L = mask_t[:, 384:768]
        pending = []
        fin_done = set()

        def drain(n):
            while len(pending) > n:
                pending.pop(0)()

        wo_parts = []  # deferred per-(st,e) Wo emission closures

        def emit_head(g, h, pstile):
            kjs = list(range(max(0, 2 * g - 8), 2 * g + 2))
            prs = [kjs[i:i + 2] for i in range(0, len(kjs), 2)]
            nb = len(prs)
            av = pstile([128, QB], "av", 2, f"av{h}_{g}")
            ptsum = None
            prev_pt = None
            for bi, pr in enumerate(prs):
                kind = 'R' if bi == nb - 1 else ('L' if bi == 0 and g >= 4 else 'P')
                # entries: (kj, col0, width, qoff); av order full-first
                if kind == 'R':       # [o=+1 right-half | o=0 full]
                    ents = [(2 * g, 256, 256, 0), (2 * g + 1, 128, 128, 128)]
                    erg = slice(128, 512)
                    zrg = slice(0, 128)
                elif kind == 'L':     # [o=-7 full | o=-8 left-half]
                    ents = [(2 * g - 7, 0, 256, 0), (2 * g - 8, 256, 128, 0)]
                    erg = slice(0, 384)
                    zrg = slice(384, 512)
                else:
                    ents = [(pr[0], 0, 256, 0), (pr[1], 256, 256, 0)]
                    erg = slice(0, 512)
                    zrg = None
                sps = pstile([128, 512], "sps", 3, f"sps{h}_{g}_{bi}")
                for kj, c0, w, qo in ents:
                    nc.tensor.matmul(
                        sps[:, c0:c0 + w],
                        lhsT=kT[:, kj * 128:(kj + 1) * 128],
                        rhs=qT[h][:, g * QB + qo:g * QB + qo + w],
                        start=True, stop=True)
                pt = ptp.tile([128, 512], dt.bfloat16, tag="pt",
                              name=f"pt{h}_{g}_{bi}")
                if zrg is not None:
                    nc.gpsimd.memset(pt[:, zrg], 0.0)
                nc.scalar.activation(
                    pt[:, erg], sps[:, erg],
                    mybir.ActivationFunctionType.Exp, scale=SCALE)
                if kind == 'R':
                    nc.vector.tensor_mul(pt[:, erg], pt[:, erg], mask_R)
                elif kind == 'L':
                    nc.vector.tensor_mul(pt[:, erg], pt[:, erg], mask_L)
                if bi == 1:
                    ptsum = smalls.tile([128, 512], dt.bfloat16, tag="ptsum",
                                        name=f"ptsum{h}_{g}")

                def mk_av(pt=pt, ents=ents, bi=bi, first=(bi == 0),
                          last=(bi == nb - 1), av=av, ptsum=ptsum,
                          prev_pt=prev_pt):
                    def f():
                        for i, (kj, c0, w, qo) in enumerate(ents):
                            nc.tensor.matmul(
                                av[:, qo:qo + w],
                                lhsT=v[:, kj * 128:(kj + 1) * 128],
                                rhs=pt[:, c0:c0 + w],
                                start=(first and i == 0),
                                stop=(last and i == len(ents) - 1))
                        # running pt-sum (softmax denominator); first
                        # add on the otherwise-idle Pool engine
                        if bi == 1:
                            nc.gpsimd.tensor_add(ptsum, prev_pt, pt)
                        elif bi > 1:
                            nc.vector.tensor_add(ptsum, ptsum, pt)
                    return f
                pending.append(mk_av())
                drain(depth)
                prev_pt = pt

            def mk_fin(h=h, g=g, av=av, ptsum=ptsum, pt=pt, pstile=pstile):
                def f():
                    # fold the two kj-halves -> per-q key-sums [128, 256]
                    src = ptsum if ptsum is not None else pt
                    ptf = smalls.tile([128, QB], dt.bfloat16, tag="ptf",
                                      name=f"ptf{h}_{g}")
                    nc.vector.tensor_add(ptf, src[:, 0:QB], src[:, QB:2 * QB])
                    denb = pstile([128, QB], "den", 1, f"den{h}_{g}")
                    nc.tensor.matmul(denb, lhsT=ones_bf, rhs=ptf,
                                     start=True, stop=True)
                    rcb = smalls.tile([128, QB], dt.float32r, tag="bcs",
                                      name=f"rcb{h}_{g}")
                    with nc.allow_low_precision(reason="f32r is full fp32 bits"):
                        nc.vector.reciprocal(rcb, denb)
                    nc.vector.tensor_mul(
                        ohT[h][:, g * QB:(g + 1) * QB], av, rcb)
                    fin_done.add((h, g))
                return f
            pending.append(mk_fin())

        # ---- Phase 1: projections (per 512-wide s-chunk) ----
        with tc.tile_pool(name="psA", bufs=8, space="PSUM") as psA:
            hst_tiles = []
            for ch in range(4):
                hst = hstp.tile([128, KT * 512], dt.bfloat16, tag="hst", name=f"hst{ch}")
                hst_tiles.append(hst)
            # chunk 0: quarter-granularity DMAs interleaved with weight quarters
            h0v = hst_tiles[0].rearrange("p (t n) -> p t n", t=KT)
            for sl in (slice(0, 1), slice(1, 2), slice(2, 4), slice(4, 7),
                       slice(7, 11), slice(11, 16)):
                nc.sync.dma_start(out=wqkv_v[:, sl], in_=wqkv[sl].rearrange("t p n -> p t n"))
                nc.sync.dma_start(out=h0v[:, sl], in_=hsT[0, sl].rearrange("t p n -> p t n"))

            for ch in range(4):
                if ch + 1 < 4:
                    nxt = hst_tiles[ch + 1]
                    nc.sync.dma_start(
                        out=nxt.rearrange("p (t n) -> p t n", t=KT),
                        in_=hsT[ch + 1].rearrange("t p n -> p t n"))
                if ch == 0:
                    nc.sync.dma_start(out=mask_t, in_=masks[:, :])
                    nc.sync.dma_start(
                        out=wo_sb.rearrange("p (c n) -> p c n", c=4),
                        in_=wo[:].rearrange("c p n -> p c n"))
                hst = hst_tiles[ch]
                q_ps = [psA.tile([128, 512], dt.float32, tag="ps", name=f"qps{ch}_{h}")
                        for h in range(GH)]
                k_ps = psA.tile([128, 512], dt.float32, tag="ps")
                v_ps = psA.tile([128, 512], dt.float32, tag="ps")

                def vtrans(ch, vt, pool=None):
                    # transpose V of a finished chunk, interleaved into the
                    # next chunk's matmul stream so PE never waits on it
                    for j in range(4):
                        if pool is None:
                            tp = psA.tile([128, 128], dt.float32r, tag="ps",
                                          name=f"tp{ch}_{j}")
                        else:
                            tp = pool.tile([128, 128], dt.float32r, tag="sps",
                                           bufs=3, name=f"tp{ch}_{j}")
                        nc.tensor.transpose(tp, vt[:, j * 128:(j + 1) * 128], ident)
                        nc.scalar.copy(
                            v[:, (4 * ch + j) * 128:(4 * ch + j + 1) * 128], tp)

                for t in range(KT):
                    if ch > 0 and t == 4:
                        vtrans(ch - 1, prev_vt)
                    rhs = hst[:, t * 512:(t + 1) * 512]
                    st = (t == 0)
                    sp = (t == KT - 1)
                    for h in range(GH):
                        nc.tensor.matmul(
                            q_ps[h], lhsT=wqkv_sb[:, t * 768 + h * 128: t * 768 + (h + 1) * 128],
                            rhs=rhs, start=st, stop=sp)
                    nc.tensor.matmul(k_ps, lhsT=wqkv_sb[:, t * 768 + 512: t * 768 + 640],
                                     rhs=rhs, start=st, stop=sp)
                    nc.tensor.matmul(v_ps, lhsT=wqkv_sb[:, t * 768 + 640: t * 768 + 768],
                                     rhs=rhs, start=st, stop=sp)
                # drain PSUM on three engines in slot-rotation order
                cs = slice(ch * 512, (ch + 1) * 512)
                nc.vector.tensor_copy(qT[0][:, cs], q_ps[0])
                nc.scalar.copy(qT[1][:, cs], q_ps[1])
                nc.vector.tensor_copy(qT[2][:, cs], q_ps[2])
                nc.vector.tensor_copy(qT[3][:, cs], q_ps[3])
                nc.scalar.copy(kT[:, cs], k_ps)
                vt = vtp.tile([128, 512], dt.float32r, tag="vt")
                nc.scalar.copy(vt, v_ps)
                prev_vt = vt

        # ---- Phase 2+3: banded attention (S^T[k,q] layout) + interleaved Wo ----
        with tc.tile_pool(name="psB", bufs=1, space="PSUM") as psB:
            def psb_tile(shape, tag, bufs, name):
                return psB.tile(shape, dt.float32, tag=tag, bufs=bufs, name=name)

            vtrans(3, prev_vt, pool=psB)

            def mk_wo(st, e, osb):
                def f():
                    wop = psB.tile([128, 512], dt.float32, tag="wop", bufs=2,
                                   name=f"wop{st}_{e}")
                    for ct in range(4):
                        nc.tensor.matmul(
                            wop, lhsT=ohT[ct][:, st * 128:(st + 1) * 128],
                            rhs=wo_sb[:, ct * 2048 + e * 512: ct * 2048 + (e + 1) * 512],
                            start=(ct == 0), stop=(ct == 3))
                    nc.scalar.copy(osb[:, e * 512:(e + 1) * 512], wop)
                    nc.sync.dma_start(
                        out=out[st, e], in_=osb[:, e * 512:(e + 1) * 512])
                return f

            def queue_wo(g):
                for st in (2 * g, 2 * g + 1):
                    osb = outp.tile([128, 2048], dt.bfloat16, tag="osb", name=f"osb{st}")
                    for e in range(4):
                        wo_parts.append(mk_wo(st, e, osb))

            for g in range(NG):
                if g >= 1:
                    queue_wo(g - 1)
                npop = 2
                for h in range(GH):
                    emit_head(g, h, psb_tile)
                    # interleave Wo pieces of earlier groups; their ohT
                    # inputs must have been written (fins emitted) first
                    if g >= 1:
                        while (3, g - 1) not in fin_done and pending:
                            pending.pop(0)()
                        for _ in range(npop):
                            if wo_parts:
                                wo_parts.pop(0)()
            drain(0)
            queue_wo(NG - 1)
            while wo_parts:
                wo_parts.pop(0)()

    nc.compile()
    return nc


def _build_masks():
    kk = np.arange(128)[:, None]
    qq = np.arange(256)[None, :]
    cc = np.arange(128)[None, :]
    # mask_R covers pt cols [128:512] of an R quad: [o=+1 right half | o=0 full]
    r1 = (kk <= cc).astype(np.float32)             # o = +1 on q in [128:256)
    r0 = (kk <= qq).astype(np.float32)             # o = 0
    # mask_L covers pt cols [0:384] of an L quad: [o=-7 full | o=-8 left half]
    l1 = (kk + 128 >= qq).astype(np.float32)       # o = -7
    l0 = (kk >= cc).astype(np.float32)             # o = -8 on q in [0:128)
    return np.hstack([r1, r0, l1, l0]).astype(BF16)  # [128, 768]


def kernel(hidden_states, Wq, Wk, Wv, Wo):
    global _nc_cache
    if _nc_cache is None:
        _nc_cache = _build_nc()
    nc = _nc_cache

    masks = _build_masks()
    hsT = []
    for b in range(B):
        ht = np.ascontiguousarray(hidden_states[b].T)                 # [H, S]
        t4 = ht.reshape(KT, 128, 4, 512).transpose(2, 0, 1, 3)        # [ch, t, 128, 512]
        hsT.append(np.ascontiguousarray(t4).astype(BF16))
    in_maps = []
    for b in range(B):
        for gi in range(KV_HEADS):
            wqkv = np.concatenate(
                [Wq[:, gi * GD:(gi + 1) * GD], Wk[:, gi * D:(gi + 1) * D],
                 Wv[:, gi * D:(gi + 1) * D]], axis=1)
            in_maps.append({
                "hsT": hsT[b],
                "wqkv": wqkv.reshape(KT, 128, GD + 2 * D).astype(BF16),
                "wo": Wo[gi * GD:(gi + 1) * GD, :].reshape(4, 128, H).astype(BF16),
                "masks": masks,
            })
    res = run_bass_kernel_spmd(nc, in_maps, list(range(8)))
    out = np.zeros((B, S, H), np.float32)
    for b in range(B):
        acc = None
        for gi in range(KV_HEADS):
            o = np.asarray(res.results[b * KV_HEADS + gi]["out"]).astype(np.float32)
            acc = o if acc is None else acc + o
        out[b] = acc.transpose(0, 2, 1, 3).reshape(S, H)              # [16,4,128,512] -> [S,H]
    return out
